# revision 26
# baseline (speedup 1.0000x reference)
"""Multi-head attention (B=4, S=2048, D=1024, H=16, Hd=64) on 8 TRN2 NeuronCores.

Sharding: tensor-parallel over heads — 2 heads per core (128 channels).
Each core computes its heads' Q/K/V projections and attention, producing the
UNNORMALIZED attention output plus softmax denominators (the ones-column
trick); the host normalizes, applies the output projection Wo per head-slice,
and sums the 8 partials + bo. Device work is ACT(exp)-bound; keeping the
output projection off-device removes the per-block finalize serialization.

Device-side structure (per core):
  - x is pre-transposed on host to xT [D, B*S]; streamed per 512-row q-tile
    for chunk 0 (earliest possible attention start) and per 2048-row chunk
    afterwards.
  - Q, K produced transposed: QT/KT [128ch, B*S], heads stacked on
    partitions (head0 rows 0:64, head1 rows 64:128). The two heads' K=64
    score matmuls are emitted adjacently at disjoint row groups, so they
    run concurrently in the 128x128 PE array.
  - V is computed transposed (VT, N=512 matmuls) then PE-transposed into
    natural [seq, ch] layout with a ones-column per head; the attention
    output matmul OT[65, q] = V_aug.T @ P carries the softmax denominator
    in row 64 for free.
  - Both heads' score tiles share one 2-bank PSUM tile, so exp() runs as
    a single 1024-wide ACT op.
  - Projections are a lazily-driven generator pulled from inside the
    attention loop, so projection matmuls (pure PE) fill the PE idle left
    by ACT-paced attention. Chunk-0 units are ordered K0,Q0 first so the
    first exp can issue ~6us into the kernel.
  - No max-subtraction in softmax: scores ~ N(0,1) by construction
    (|score| < ~7), exp() is safe in fp32.
"""
import sys

sys.path.insert(0, "/opt/trn_rl_repo")

import numpy as np
import ml_dtypes

import concourse.bass as bass
import concourse.mybir as mybir
import concourse.tile as tile
from concourse import bacc, bass_utils
from concourse.masks import make_identity

B, S, D = 4, 2048, 1024
BS = B * S            # 8192 rows
NCORES = 8
CPC = 128             # channels per core (2 heads x 64)
HD = 64               # head dim
P = 128
QT_TILE = 512         # q-tile width
NQT = BS // QT_TILE   # 16
NKT = S // P          # 16 k-tiles per batch
NQA = S // QT_TILE    # 4 q-tiles per batch
OC = 2 * HD + 2       # 130: [o_h0(64); d_h0; o_h1(64); d_h1] output channels

F32 = mybir.dt.float32
CD = mybir.dt.bfloat16          # compute dtype on device
CD_NP = ml_dtypes.bfloat16

LAST_RESULTS = None
_NC_CACHE = {}


def build_nc():
    if "nc" in _NC_CACHE:
        return _NC_CACHE["nc"]
    nc = bacc.Bacc(trn_type="TRN2", num_devices=NCORES)

    # x pre-laid-out on host as [p, qtile, o, qcol] so every DMA slice is
    # 8KB-contiguous per partition (full-rate descriptors)
    x3 = nc.dram_tensor("x3", [P, NQT, D // P, QT_TILE], CD, kind="ExternalInput").ap()
    wq = nc.dram_tensor("wq", [P, D // P, CPC], CD, kind="ExternalInput").ap()
    wk = nc.dram_tensor("wk", [P, D // P, CPC], CD, kind="ExternalInput").ap()
    wv = nc.dram_tensor("wv", [P, D // P, CPC], CD, kind="ExternalInput").ap()
    bq = nc.dram_tensor("bq", [CPC, 1], F32, kind="ExternalInput").ap()
    bk = nc.dram_tensor("bk", [CPC, 1], F32, kind="ExternalInput").ap()
    bv = nc.dram_tensor("bv", [CPC, 1], F32, kind="ExternalInput").ap()
    o_out = nc.dram_tensor("o_out", [OC, BS], F32, kind="ExternalOutput").ap()

    KCH = D // P  # 8 contraction chunks for the projections
    scale = float(1.0 / np.sqrt(np.float32(HD)))

    with tile.TileContext(nc) as tc:
        with (
            tc.tile_pool(name="pers", bufs=1) as pers,
            tc.tile_pool(name="xin", bufs=2) as xin,
            tc.tile_pool(name="vtp", bufs=2) as vtp,
            tc.tile_pool(name="pt", bufs=4) as pt,
            tc.tile_pool(name="otn", bufs=4) as otn_pool,
            tc.tile_pool(name="psW", bufs=2, space="PSUM") as psW,
            tc.tile_pool(name="psOT", bufs=2, space="PSUM") as psOT,
            tc.tile_pool(name="ps2", bufs=2, space="PSUM") as ps2,
        ):
            # ---- persistent tensors ----
            qt_sb = pers.tile([P, BS], CD, tag="QT")
            kt_sb = pers.tile([P, BS], CD, tag="KT")
            v_sb = pers.tile([P, BS // P, OC], CD, tag="V")
            wq_sb = pers.tile([P, KCH, CPC], CD, tag="wq")
            wk_sb = pers.tile([P, KCH, CPC], CD, tag="wk")
            wv_sb = pers.tile([P, KCH, CPC], CD, tag="wv")
            bq_sb = pers.tile([CPC, 1], F32, tag="bq")
            bk_sb = pers.tile([CPC, 1], F32, tag="bk")
            bv_sb = pers.tile([CPC, 1], F32, tag="bv")
            ident_sb = pers.tile([P, P], CD, tag="ident")
            warm_sb = pers.tile([P, QT_TILE], CD, tag="warm")

            make_identity(nc, ident_sb[:])
            nc.vector.memset(warm_sb[:], 0.0)

            # ---- phase 1: projections, as a lazily-driven generator ----
            # Units are pulled from inside the attention loop so projection
            # matmuls (pure PE) fill the PE idle left by ACT-paced attention.
            XQ = BS // 4  # 2048 rows per x chunk

            # Projection sub-units are HALF accumulation runs (4 matmuls,
            # ~0.9us) so a pull landing between two score pairs can never
            # delay the next exp by more than ~1us. The two halves of a run
            # share one psW tile; pull discipline in the attention loop
            # guarantees no other psW allocation lands between them (which
            # would rotate the 2-slot pool onto the live accumulator).
            def emit_proj_half(xt_lq, w_sb, half, pj):
                for o in range(half * 4, half * 4 + 4):
                    nc.tensor.matmul(
                        pj[:], w_sb[:, o, :], xt_lq[:, o, :],
                        start=(o == 0), stop=(o == KCH - 1),
                    )

            def emit_v_transpose_half(vt_sb, q0, half):
                for rt in (2 * half, 2 * half + 1):
                    tp = psW.tile([P, P], CD, tag="w", name="tp")
                    nc.tensor.transpose(
                        tp[:], vt_sb[:, rt * P : (rt + 1) * P], ident_sb[:]
                    )
                    grt = q0 // P + rt
                    nc.vector.tensor_copy(out=v_sb[:, grt, 0:HD], in_=tp[:, 0:HD])
                    nc.vector.tensor_copy(
                        out=v_sb[:, grt, HD + 1 : 2 * HD + 1], in_=tp[:, HD:CPC]
                    )
                    nc.vector.memset(v_sb[:, grt, HD : HD + 1], 1.0)
                    nc.vector.memset(v_sb[:, grt, 2 * HD + 1 : OC], 1.0)

            # chunk-0 x tile; its first slice leads the DMA queue (split in
            # two so K0's first half-run can start after half the transfer),
            # followed by the weights in first-use order. Every DMA issue
            # costs ~650ns on the sync engine, so order matters here.
            xt0 = xin.tile([P, XQ // QT_TILE, KCH, QT_TILE], CD, tag="xt")
            for qtr in range(4):
                nc.sync.dma_start(
                    xt0[:, 0, 2 * qtr : 2 * qtr + 2], x3[:, 0, 2 * qtr : 2 * qtr + 2]
                )
            nc.sync.dma_start(wk_sb[:], wk[:, :, :])
            nc.sync.dma_start(wq_sb[:], wq[:, :, :])
            nc.sync.dma_start(bk_sb[:], bk[:, :])
            nc.sync.dma_start(bq_sb[:], bq[:, :])
            nc.sync.dma_start(wv_sb[:], wv[:, :, :])
            nc.sync.dma_start(bv_sb[:], bv[:, :])

            # ~3us of dummy matmuls with no DMA dependency: warms the PE HAM
            # clock gate (cold K=4/8 -> warm 8/8 takes ~3.4us of activity)
            # while the first x slice is still in flight on the DMA engines.
            # Each result is read by a 1-element scalar copy so the matmuls
            # are not dead code.
            for wi in range(10):
                wm = psW.tile([P, QT_TILE], F32, tag="w", name="wm")
                nc.tensor.matmul(
                    wm[:, 0:P], warm_sb[:, 0:P], warm_sb[:, 0:P],
                    start=True, stop=True,
                )
                nc.scalar.copy(warm_sb[0:1, wi : wi + 1], wm[0:1, 0:1])

            def proj_gen():
                # chunk 0: per-512-row slices, ordered so attention on
                # (0,0) can start after just [K0, Q0]; V and the later
                # K/Q tiles stream in as filler pulled by the attention loop.
                def proj_sub(xt_t, lq, w_sb, b_sb, dst, q0, dma=None):
                    pj = psW.tile([P, QT_TILE], F32, tag="w", name="pj")
                    if dma is not None:
                        dma()
                    emit_proj_half(xt_t[:, lq], w_sb, 0, pj)
                    yield
                    emit_proj_half(xt_t[:, lq], w_sb, 1, pj)
                    nc.vector.tensor_scalar_add(
                        dst[:, q0 : q0 + QT_TILE], pj[:], b_sb[:, 0:1]
                    )
                    yield

                def v_sub(xt_t, lq, q0):
                    vt = vtp.tile([P, QT_TILE], CD, tag="vt")
                    pj = psW.tile([P, QT_TILE], F32, tag="w", name="pj")
                    emit_proj_half(xt_t[:, lq], wv_sb, 0, pj)
                    yield
                    emit_proj_half(xt_t[:, lq], wv_sb, 1, pj)
                    nc.vector.tensor_scalar_add(vt[:], pj[:], bv_sb[:, 0:1])
                    yield
                    emit_v_transpose_half(vt, q0, 0)
                    yield
                    emit_v_transpose_half(vt, q0, 1)
                    yield

                def chunk0_subs(lq):
                    def dma():
                        if lq > 0:
                            nc.sync.dma_start(xt0[:, lq], x3[:, lq])
                    k = proj_sub(xt0, lq, wk_sb, bk_sb, kt_sb, lq * QT_TILE, dma)
                    q = proj_sub(xt0, lq, wq_sb, bq_sb, qt_sb, lq * QT_TILE)
                    v = v_sub(xt0, lq, lq * QT_TILE)
                    return k, q, v

                # order: k0,q0 (pre-pulled), then V/K interleaved so block
                # (0,0)'s progressive kt needs are met, Q1-3 at the end
                subs0 = []
                k0, q0u, v0 = chunk0_subs(0)
                subs0 += [k0, k0, q0u, q0u, v0, v0, v0, v0]
                k1, q1u, v1 = chunk0_subs(1)
                subs0 += [k1, k1, v1, v1, v1, v1]
                k2, q2u, v2 = chunk0_subs(2)
                subs0 += [k2, k2, v2, v2, v2, v2]
                k3, q3u, v3 = chunk0_subs(3)
                subs0 += [k3, k3, v3, v3, v3, v3]
                subs0 += [q1u, q1u, q2u, q2u, q3u, q3u]
                for s in subs0:
                    next(s)
                    yield

                # chunks 1-3: one big DMA per chunk, then per-512 q/k/v subs
                for xq in range(1, 4):
                    xt = xin.tile([P, XQ // QT_TILE, KCH, QT_TILE], CD, tag="xt")
                    nc.sync.dma_start(xt[:], x3[:, xq * 4 : (xq + 1) * 4])
                    yield
                    for lq in range(XQ // QT_TILE):
                        q0 = xq * XQ + lq * QT_TILE
                        for s in (
                            proj_sub(xt, lq, wq_sb, bq_sb, qt_sb, q0),
                            proj_sub(xt, lq, wk_sb, bk_sb, kt_sb, q0),
                            v_sub(xt, lq, q0),
                        ):
                            for _ in s:
                                yield

            gen = proj_gen()
            pulled = [0]
            pull_cap = [10**9]

            def pull(n):
                for _ in range(n):
                    if pulled[0] >= pull_cap[0]:
                        break
                    if next(gen, "done") == "done":
                        break
                    pulled[0] += 1

            UNITS0 = 32
            UNITS_PER_CHUNK = 33
            TOTAL_UNITS = UNITS0 + 3 * UNITS_PER_CHUNK  # 131
            NEED = [UNITS0, UNITS0 + 33, UNITS0 + 66, TOTAL_UNITS]

            pull(4)  # K0, Q0 — enough for the first score matmuls

            # ---- phase 2: attention ----
            def emit_st_exp(b, qa, kt):
                q0 = b * S + qa * QT_TILE
                k0 = b * S + kt * P
                stp = ps2.tile([P, 2 * QT_TILE], F32, tag="stp", name="stp")
                for h in range(2):
                    hp = h * HD
                    nc.tensor.matmul(
                        stp[:, h * QT_TILE : (h + 1) * QT_TILE],
                        kt_sb[hp : hp + HD, k0 : k0 + P],
                        qt_sb[hp : hp + HD, q0 : q0 + QT_TILE],
                        start=True, stop=True,
                    )
                p_t = pt.tile([P, 2 * QT_TILE], CD, tag="p", name="p")
                nc.scalar.activation(
                    p_t[:], stp[:], mybir.ActivationFunctionType.Exp, scale=scale
                )
                return p_t

            def emit_av_kt(ot, b, kt, p_t):
                # one k-tile's AV for both heads: everything here becomes
                # ready at exp(kt)-end, so it can sit in the PE FIFO right
                # after a score pair gated by the same event
                for h in range(2):
                    vcol = h * (HD + 1)
                    nc.tensor.matmul(
                        ot[h][0 : HD + 1, :],
                        v_sb[:, b * NKT + kt, vcol : vcol + HD + 1],
                        p_t[:, h * QT_TILE : (h + 1) * QT_TILE],
                        start=(kt == 0), stop=(kt == NKT - 1),
                    )

            def emit_tail_av(tail, kt):
                # one k-tile of the previous block's last AV group, emitted
                # between the next block's first score pairs so it fills the
                # PE while those pairs wait for their PSUM banks
                tb, tot, tpts, tq0 = tail
                emit_av_kt(tot, tb, kt, tpts[kt])

            def emit_tail_evac(tail):
                tb, tot, tpts, tq0 = tail
                for h in range(2):
                    otu = otn_pool.tile(
                        [HD + 1, QT_TILE], F32, tag="otu", name=f"otu{h}"
                    )
                    nc.vector.tensor_copy(out=otu[:], in_=tot[h][0 : HD + 1, :])
                    nc.sync.dma_start(
                        o_out[h * (HD + 1) : (h + 1) * (HD + 1), tq0 : tq0 + QT_TILE],
                        otu[:],
                    )

            blocks = [(b, qa) for b in range(B) for qa in range(NQA)]
            tail = None
            for bi, (b, qa) in enumerate(blocks):
                # all of batch b's projections must be emitted before its
                # attention reads them (deps are traced in emission order);
                # batch 3's tail (its last two q-tiles) is deliberately
                # withheld and fed into block (3,0)'s kt loop below, so the
                # projection-less final batch still has PE filler.
                if (b, qa) == (0, 0):
                    need = 4
                elif (b, qa) == (3, 0):
                    need = TOTAL_UNITS - 16
                else:
                    need = NEED[b]
                pull_cap[0] = TOTAL_UNITS if b >= 3 else TOTAL_UNITS - 16
                deficit = min(need, pull_cap[0]) - pulled[0]
                if deficit > 0:
                    pull(deficit)
                # soft pacing target: pull ahead only toward the next batch's
                # requirement, so projections neither bunch up between exps
                # nor front-load the PE
                soft = min(pull_cap[0], NEED[b + 1] if b < 3 else TOTAL_UNITS)

                def pull_soft(n):
                    pull(min(n, max(0, soft - pulled[0])))

                pts = {0: emit_st_exp(b, qa, 0)}
                if tail is not None:
                    emit_tail_av(tail, NKT - 2)
                pts[1] = emit_st_exp(b, qa, 1)
                if tail is not None:
                    emit_tail_av(tail, NKT - 1)
                    emit_tail_evac(tail)
                ot = [
                    psOT.tile([P, QT_TILE], F32, tag="ot", name=f"ot{h}")
                    for h in range(2)
                ]
                for kt in range(2, NKT, 2):
                    if (b, qa) == (0, 0):
                        # block (0,0) is projection-bound: its AV(kt) needs
                        # chunk-0's V subs complete just-in-time, which only
                        # a pull-at-top schedule satisfies
                        pull(4)
                        pts[kt] = emit_st_exp(b, qa, kt)
                        pts[kt + 1] = emit_st_exp(b, qa, kt + 1)
                        emit_av_kt(ot, b, kt - 2, pts.pop(kt - 2))
                        emit_av_kt(ot, b, kt - 1, pts.pop(kt - 1))
                        continue
                    # the exp-gating chain is exp(kt-2) -> scores(kt) ->
                    # exp(kt): each emission after a score pair becomes ready
                    # at the same exp-completion that pair waits on, so the
                    # FIFO fills the waits instead of extending them; no pull
                    # after the last scores of a block (it would push out the
                    # next block's first pair)
                    special = (b, qa) == (3, 0)
                    pts[kt] = emit_st_exp(b, qa, kt)
                    emit_av_kt(ot, b, kt - 2, pts.pop(kt - 2))
                    pull(1) if special else pull_soft(1)
                    pts[kt + 1] = emit_st_exp(b, qa, kt + 1)
                    emit_av_kt(ot, b, kt - 1, pts.pop(kt - 1))
                    if special:
                        pull(1)
                    if kt < NKT - 2:
                        pull(1) if special else pull_soft(1)
                tail = (b, ot, {NKT - 2: pts.pop(NKT - 2), NKT - 1: pts.pop(NKT - 1)},
                        b * S + qa * QT_TILE)
            emit_tail_av(tail, NKT - 2)
            emit_tail_av(tail, NKT - 1)
            emit_tail_evac(tail)

    nc.compile()
    _NC_CACHE["nc"] = nc
    return nc


def make_in_maps(inputs):
    x = np.asarray(inputs["x"], np.float32)
    Wq = np.asarray(inputs["Wq"], np.float32)
    Wk = np.asarray(inputs["Wk"], np.float32)
    Wv = np.asarray(inputs["Wv"], np.float32)
    bq = np.asarray(inputs["bq"], np.float32)
    bk = np.asarray(inputs["bk"], np.float32)
    bv = np.asarray(inputs["bv"], np.float32)

    # x3[p, qtile, o, qcol] = x[qtile*512+qcol, o*128+p] — 8KB-contiguous
    # per-partition DMA slices
    x3 = np.ascontiguousarray(
        x.reshape(NQT, QT_TILE, D // P, P).transpose(3, 0, 2, 1)
    ).astype(CD_NP)

    def w3(W, sl):
        # w3[p, o, c] = W[o*128+p, c]
        return np.ascontiguousarray(
            W[:, sl].reshape(D // P, P, CPC).transpose(1, 0, 2)
        ).astype(CD_NP)

    in_maps = []
    for c in range(NCORES):
        sl = slice(c * CPC, (c + 1) * CPC)
        in_maps.append(
            {
                "x3": x3,
                "wq": w3(Wq, sl),
                "wk": w3(Wk, sl),
                "wv": w3(Wv, sl),
                "bq": np.ascontiguousarray(bq[sl].reshape(CPC, 1)),
                "bk": np.ascontiguousarray(bk[sl].reshape(CPC, 1)),
                "bv": np.ascontiguousarray(bv[sl].reshape(CPC, 1)),
            }
        )
    return in_maps


def host_epilogue(results, Wo, bo):
    """Normalize each core's unnormalized attention output and apply the
    output projection on host: y = sum_c (O_c / d_c).T @ Wo[c-slice] + bo."""
    on_full = np.empty((BS, D), np.float32)
    for c, r in enumerate(results):
        o = np.asarray(r["o_out"], np.float32)  # [130, BS]
        for h in range(2):
            num = o[h * (HD + 1) : h * (HD + 1) + HD, :]      # [64, BS]
            den = o[h * (HD + 1) + HD : h * (HD + 1) + HD + 1, :]  # [1, BS]
            on_full[:, (2 * c + h) * HD : (2 * c + h + 1) * HD] = (num / den).T
    y = on_full @ Wo.astype(np.float32) + bo.astype(np.float32)
    return y


def kernel(**inputs):
    global LAST_RESULTS
    Wo = np.asarray(inputs["Wo"], np.float32)
    bo = np.asarray(inputs["bo"], np.float32)
    nc = build_nc()
    in_maps = make_in_maps(inputs)
    res = bass_utils.run_bass_kernel_spmd(nc, in_maps, core_ids=list(range(NCORES)))
    LAST_RESULTS = res
    y = host_epilogue(res.results, Wo, bo)
    return y.reshape(B, S, D)


# revision 30
# speedup vs baseline: 1.0487x; 1.0487x over previous
"""Multi-head attention (B=4, S=2048, D=1024, H=16, Hd=64) on 8 TRN2 NeuronCores.

Sharding: tensor-parallel over heads — 2 heads per core (128 channels).
Each core computes its heads' Q/K/V projections and attention, producing the
UNNORMALIZED attention output plus softmax denominators (the ones-column
trick); the host normalizes, applies the output projection Wo per head-slice,
and sums the 8 partials + bo. Device work is ACT(exp)-bound; keeping the
output projection off-device removes the per-block finalize serialization.

Device-side structure (per core):
  - x is pre-transposed on host to xT [D, B*S]; streamed per 512-row q-tile
    for chunk 0 (earliest possible attention start) and per 2048-row chunk
    afterwards.
  - Q, K produced transposed: QT/KT [128ch, B*S], heads stacked on
    partitions (head0 rows 0:64, head1 rows 64:128). The two heads' K=64
    score matmuls are emitted adjacently at disjoint row groups, so they
    run concurrently in the 128x128 PE array.
  - V is computed transposed (VT, N=512 matmuls) then PE-transposed into
    natural [seq, ch] layout with a ones-column per head; the attention
    output matmul OT[65, q] = V_aug.T @ P carries the softmax denominator
    in row 64 for free.
  - Both heads' score tiles share one 2-bank PSUM tile, so exp() runs as
    a single 1024-wide ACT op.
  - Projections are a lazily-driven generator pulled from inside the
    attention loop, so projection matmuls (pure PE) fill the PE idle left
    by ACT-paced attention. Chunk-0 units are ordered K0,Q0 first so the
    first exp can issue ~6us into the kernel.
  - No max-subtraction in softmax: scores ~ N(0,1) by construction
    (|score| < ~7), exp() is safe in fp32.
"""
import sys

sys.path.insert(0, "/opt/trn_rl_repo")

import numpy as np
import ml_dtypes

import concourse.bass as bass
import concourse.mybir as mybir
import concourse.tile as tile
from concourse import bacc, bass_utils
from concourse.masks import make_identity

B, S, D = 4, 2048, 1024
BS = B * S            # 8192 rows
NCORES = 8
CPC = 128             # channels per core (2 heads x 64)
HD = 64               # head dim
P = 128
QT_TILE = 512         # q-tile width
NQT = BS // QT_TILE   # 16
NKT = S // P          # 16 k-tiles per batch
NQA = S // QT_TILE    # 4 q-tiles per batch
OC = 2 * HD + 2       # 130: [o_h0(64); d_h0; o_h1(64); d_h1] output channels

F32 = mybir.dt.float32
CD = mybir.dt.bfloat16          # compute dtype on device
CD_NP = ml_dtypes.bfloat16

LAST_RESULTS = None
_NC_CACHE = {}


def build_nc():
    if "nc" in _NC_CACHE:
        return _NC_CACHE["nc"]
    nc = bacc.Bacc(trn_type="TRN2", num_devices=NCORES)

    # x pre-laid-out on host as [p, qtile, o, qcol] so every DMA slice is
    # 8KB-contiguous per partition (full-rate descriptors)
    x3 = nc.dram_tensor("x3", [P, NQT, D // P, QT_TILE], CD, kind="ExternalInput").ap()
    wq = nc.dram_tensor("wq", [P, D // P, CPC], CD, kind="ExternalInput").ap()
    wk = nc.dram_tensor("wk", [P, D // P, CPC], CD, kind="ExternalInput").ap()
    wv = nc.dram_tensor("wv", [P, D // P, CPC], CD, kind="ExternalInput").ap()
    bq = nc.dram_tensor("bq", [CPC, 1], F32, kind="ExternalInput").ap()
    bk = nc.dram_tensor("bk", [CPC, 1], F32, kind="ExternalInput").ap()
    bv = nc.dram_tensor("bv", [CPC, 1], F32, kind="ExternalInput").ap()
    o_out = nc.dram_tensor("o_out", [OC, BS], F32, kind="ExternalOutput").ap()

    KCH = D // P  # 8 contraction chunks for the projections
    scale = float(1.0 / np.sqrt(np.float32(HD)))

    with tile.TileContext(nc) as tc:
        with (
            tc.tile_pool(name="pers", bufs=1) as pers,
            tc.tile_pool(name="xin", bufs=2) as xin,
            tc.tile_pool(name="vtp", bufs=2) as vtp,
            tc.tile_pool(name="pt", bufs=8) as pt,
            tc.tile_pool(name="otn", bufs=4) as otn_pool,
            tc.tile_pool(name="psW", bufs=2, space="PSUM") as psW,
            tc.tile_pool(name="psOT", bufs=2, space="PSUM") as psOT,
            tc.tile_pool(name="ps2", bufs=2, space="PSUM") as ps2,
        ):
            # ---- persistent tensors ----
            qt_sb = pers.tile([P, BS], CD, tag="QT")
            kt_sb = pers.tile([P, BS], CD, tag="KT")
            v_sb = pers.tile([P, BS // P, OC], CD, tag="V")
            wq_sb = pers.tile([P, KCH, CPC], CD, tag="wq")
            wk_sb = pers.tile([P, KCH, CPC], CD, tag="wk")
            wv_sb = pers.tile([P, KCH, CPC], CD, tag="wv")
            bq_sb = pers.tile([CPC, 1], F32, tag="bq")
            bk_sb = pers.tile([CPC, 1], F32, tag="bk")
            bv_sb = pers.tile([CPC, 1], F32, tag="bv")
            ident_sb = pers.tile([P, P], CD, tag="ident")
            warm_sb = pers.tile([P, QT_TILE], CD, tag="warm")

            make_identity(nc, ident_sb[:])
            nc.vector.memset(warm_sb[:], 0.0)

            # ---- phase 1: projections, as a lazily-driven generator ----
            # Units are pulled from inside the attention loop so projection
            # matmuls (pure PE) fill the PE idle left by ACT-paced attention.
            XQ = BS // 4  # 2048 rows per x chunk

            # Projection sub-units are HALF accumulation runs (4 matmuls,
            # ~0.9us) so a pull landing between two score pairs can never
            # delay the next exp by more than ~1us. The two halves of a run
            # share one psW tile; pull discipline in the attention loop
            # guarantees no other psW allocation lands between them (which
            # would rotate the 2-slot pool onto the live accumulator).
            def emit_proj_half(xt_lq, w_sb, half, pj):
                for o in range(half * 4, half * 4 + 4):
                    nc.tensor.matmul(
                        pj[:], w_sb[:, o, :], xt_lq[:, o, :],
                        start=(o == 0), stop=(o == KCH - 1),
                    )

            def emit_v_transpose_half(vt_sb, q0, half):
                for rt in (2 * half, 2 * half + 1):
                    tp = psW.tile([P, P], CD, tag="w", name="tp")
                    nc.tensor.transpose(
                        tp[:], vt_sb[:, rt * P : (rt + 1) * P], ident_sb[:]
                    )
                    grt = q0 // P + rt
                    nc.vector.tensor_copy(out=v_sb[:, grt, 0:HD], in_=tp[:, 0:HD])
                    nc.vector.tensor_copy(
                        out=v_sb[:, grt, HD + 1 : 2 * HD + 1], in_=tp[:, HD:CPC]
                    )
                    nc.vector.memset(v_sb[:, grt, HD : HD + 1], 1.0)
                    nc.vector.memset(v_sb[:, grt, 2 * HD + 1 : OC], 1.0)

            # chunk-0 x tile; its first slice leads the DMA queue (split in
            # two so K0's first half-run can start after half the transfer),
            # followed by the weights in first-use order. Every DMA issue
            # costs ~650ns on the sync engine, so order matters here.
            xt0 = xin.tile([P, XQ // QT_TILE, KCH, QT_TILE], CD, tag="xt")
            for qtr in range(4):
                nc.sync.dma_start(
                    xt0[:, 0, 2 * qtr : 2 * qtr + 2], x3[:, 0, 2 * qtr : 2 * qtr + 2]
                )
            nc.sync.dma_start(wk_sb[:], wk[:, :, :])
            nc.sync.dma_start(wq_sb[:], wq[:, :, :])
            nc.sync.dma_start(bk_sb[:], bk[:, :])
            nc.sync.dma_start(bq_sb[:], bq[:, :])
            nc.sync.dma_start(wv_sb[:], wv[:, :, :])
            nc.sync.dma_start(bv_sb[:], bv[:, :])

            # ~3us of dummy matmuls with no DMA dependency: warms the PE HAM
            # clock gate (cold K=4/8 -> warm 8/8 takes ~3.4us of activity)
            # while the first x slice is still in flight on the DMA engines.
            # Each result is read by a 1-element scalar copy so the matmuls
            # are not dead code.
            for wi in range(10):
                wm = psW.tile([P, QT_TILE], F32, tag="w", name="wm")
                nc.tensor.matmul(
                    wm[:, 0:P], warm_sb[:, 0:P], warm_sb[:, 0:P],
                    start=True, stop=True,
                )
                nc.scalar.copy(warm_sb[0:1, wi : wi + 1], wm[0:1, 0:1])

            def proj_gen():
                # chunk 0: per-512-row slices, ordered so attention on
                # (0,0) can start after just [K0, Q0]; V and the later
                # K/Q tiles stream in as filler pulled by the attention loop.
                def proj_sub(xt_t, lq, w_sb, b_sb, dst, q0, dma=None):
                    pj = psW.tile([P, QT_TILE], F32, tag="w", name="pj")
                    if dma is not None:
                        dma()
                    emit_proj_half(xt_t[:, lq], w_sb, 0, pj)
                    yield
                    emit_proj_half(xt_t[:, lq], w_sb, 1, pj)
                    nc.vector.tensor_scalar_add(
                        dst[:, q0 : q0 + QT_TILE], pj[:], b_sb[:, 0:1]
                    )
                    yield

                def v_sub(xt_t, lq, q0):
                    vt = vtp.tile([P, QT_TILE], CD, tag="vt")
                    pj = psW.tile([P, QT_TILE], F32, tag="w", name="pj")
                    emit_proj_half(xt_t[:, lq], wv_sb, 0, pj)
                    yield
                    emit_proj_half(xt_t[:, lq], wv_sb, 1, pj)
                    nc.vector.tensor_scalar_add(vt[:], pj[:], bv_sb[:, 0:1])
                    yield
                    emit_v_transpose_half(vt, q0, 0)
                    yield
                    emit_v_transpose_half(vt, q0, 1)
                    yield

                def chunk0_subs(lq):
                    def dma():
                        if lq > 0:
                            nc.sync.dma_start(xt0[:, lq], x3[:, lq])
                    k = proj_sub(xt0, lq, wk_sb, bk_sb, kt_sb, lq * QT_TILE, dma)
                    q = proj_sub(xt0, lq, wq_sb, bq_sb, qt_sb, lq * QT_TILE)
                    v = v_sub(xt0, lq, lq * QT_TILE)
                    return k, q, v

                # order: k0,q0 (pre-pulled), then V/K interleaved so block
                # (0,0)'s progressive kt needs are met, Q1-3 at the end
                subs0 = []
                k0, q0u, v0 = chunk0_subs(0)
                subs0 += [k0, k0, q0u, q0u, v0, v0, v0, v0]
                k1, q1u, v1 = chunk0_subs(1)
                subs0 += [k1, k1, v1, v1, v1, v1]
                k2, q2u, v2 = chunk0_subs(2)
                subs0 += [k2, k2, v2, v2, v2, v2]
                k3, q3u, v3 = chunk0_subs(3)
                subs0 += [k3, k3, v3, v3, v3, v3]
                subs0 += [q1u, q1u, q2u, q2u, q3u, q3u]
                for s in subs0:
                    next(s)
                    yield

                # chunks 1-3: one big DMA per chunk, then per-512 q/k/v subs
                for xq in range(1, 4):
                    xt = xin.tile([P, XQ // QT_TILE, KCH, QT_TILE], CD, tag="xt")
                    nc.sync.dma_start(xt[:], x3[:, xq * 4 : (xq + 1) * 4])
                    yield
                    for lq in range(XQ // QT_TILE):
                        q0 = xq * XQ + lq * QT_TILE
                        for s in (
                            proj_sub(xt, lq, wq_sb, bq_sb, qt_sb, q0),
                            proj_sub(xt, lq, wk_sb, bk_sb, kt_sb, q0),
                            v_sub(xt, lq, q0),
                        ):
                            for _ in s:
                                yield

            gen = proj_gen()
            pulled = [0]
            pull_cap = [10**9]

            def pull(n):
                for _ in range(n):
                    if pulled[0] >= pull_cap[0]:
                        break
                    if next(gen, "done") == "done":
                        break
                    pulled[0] += 1

            UNITS0 = 32
            UNITS_PER_CHUNK = 33
            TOTAL_UNITS = UNITS0 + 3 * UNITS_PER_CHUNK  # 131
            NEED = [UNITS0, UNITS0 + 33, UNITS0 + 66, TOTAL_UNITS]

            pull(4)  # K0, Q0 — enough for the first score matmuls

            # ---- phase 2: attention ----
            def emit_st_exp(b, qa, kt):
                q0 = b * S + qa * QT_TILE
                k0 = b * S + kt * P
                stp = ps2.tile([P, 2 * QT_TILE], F32, tag="stp", name="stp")
                for h in range(2):
                    hp = h * HD
                    nc.tensor.matmul(
                        stp[:, h * QT_TILE : (h + 1) * QT_TILE],
                        kt_sb[hp : hp + HD, k0 : k0 + P],
                        qt_sb[hp : hp + HD, q0 : q0 + QT_TILE],
                        start=True, stop=True,
                    )
                p_t = pt.tile([P, 2 * QT_TILE], CD, tag="p", name="p")
                nc.scalar.activation(
                    p_t[:], stp[:], mybir.ActivationFunctionType.Exp, scale=scale
                )
                return p_t

            def emit_av_kt(ot, b, kt, p_t):
                # one k-tile's AV for both heads: everything here becomes
                # ready at exp(kt)-end, so it can sit in the PE FIFO right
                # after a score pair gated by the same event
                for h in range(2):
                    vcol = h * (HD + 1)
                    nc.tensor.matmul(
                        ot[h][0 : HD + 1, :],
                        v_sb[:, b * NKT + kt, vcol : vcol + HD + 1],
                        p_t[:, h * QT_TILE : (h + 1) * QT_TILE],
                        start=(kt == 0), stop=(kt == NKT - 1),
                    )

            # AV runs LAG k-tiles behind the exp stream (p tiles buffered in
            # the 8-deep pt pool), so score pairs are always at the PE FIFO
            # head when their PSUM bank frees — AV and projection pulls drain
            # in the slack behind them and never gate the next exp.
            pend = []

            def drain_av(n):
                for _ in range(n):
                    if not pend:
                        return
                    ot_, b_, kt_, p_, q0_ = pend.pop(0)
                    emit_av_kt(ot_, b_, kt_, p_)
                    if kt_ == NKT - 1:
                        for h in range(2):
                            otu = otn_pool.tile(
                                [HD + 1, QT_TILE], F32, tag="otu", name=f"otu{h}"
                            )
                            nc.vector.tensor_copy(
                                out=otu[:], in_=ot_[h][0 : HD + 1, :]
                            )
                            nc.sync.dma_start(
                                o_out[
                                    h * (HD + 1) : (h + 1) * (HD + 1),
                                    q0_ : q0_ + QT_TILE,
                                ],
                                otu[:],
                            )

            LAG = 4
            blocks = [(b, qa) for b in range(B) for qa in range(NQA)]
            for bi, (b, qa) in enumerate(blocks):
                # all of batch b's projections must be emitted before its
                # attention reads them (deps are traced in emission order);
                # batch 3's tail (its last two q-tiles) is deliberately
                # withheld and fed into block (3,0)'s kt loop below, so the
                # projection-less final batch still has PE filler.
                if (b, qa) == (0, 0):
                    need = 4
                elif (b, qa) == (3, 0):
                    need = TOTAL_UNITS - 16
                else:
                    need = NEED[b]
                pull_cap[0] = TOTAL_UNITS if b >= 3 else TOTAL_UNITS - 16
                deficit = min(need, pull_cap[0]) - pulled[0]
                if deficit > 0:
                    pull(deficit)
                # soft pacing target: pull ahead only toward the next batch's
                # requirement, so projections neither bunch up between exps
                # nor front-load the PE
                soft = min(pull_cap[0], NEED[b + 1] if b < 3 else TOTAL_UNITS)

                def pull_soft(n):
                    pull(min(n, max(0, soft - pulled[0])))

                ot = [
                    psOT.tile([P, QT_TILE], F32, tag="ot", name=f"ot{h}")
                    for h in range(2)
                ]
                q0 = b * S + qa * QT_TILE
                for kt in range(0, NKT, 2):
                    if (b, qa) == (0, 0) and kt >= 2:
                        # block (0,0) is projection-bound: its scores need
                        # chunk-0's K subs complete just-in-time, which only
                        # a pull-at-top schedule satisfies
                        pull(4)
                    p0 = emit_st_exp(b, qa, kt)
                    p1 = emit_st_exp(b, qa, kt + 1)
                    pend.append((ot, b, kt, p0, q0))
                    pend.append((ot, b, kt + 1, p1, q0))
                    while len(pend) > LAG + 2:
                        drain_av(1)
                    special = (b, qa) in ((0, 0), (3, 0))
                    if special:
                        if (b, qa) == (3, 0):
                            pull(3)
                    else:
                        pull_soft(1)
                        if kt < NKT - 2 or qa == NQA - 1:
                            pull_soft(1)
            while pend:
                drain_av(1)

    nc.compile()
    _NC_CACHE["nc"] = nc
    return nc


def make_in_maps(inputs):
    x = np.asarray(inputs["x"], np.float32)
    Wq = np.asarray(inputs["Wq"], np.float32)
    Wk = np.asarray(inputs["Wk"], np.float32)
    Wv = np.asarray(inputs["Wv"], np.float32)
    bq = np.asarray(inputs["bq"], np.float32)
    bk = np.asarray(inputs["bk"], np.float32)
    bv = np.asarray(inputs["bv"], np.float32)

    # x3[p, qtile, o, qcol] = x[qtile*512+qcol, o*128+p] — 8KB-contiguous
    # per-partition DMA slices
    x3 = np.ascontiguousarray(
        x.reshape(NQT, QT_TILE, D // P, P).transpose(3, 0, 2, 1)
    ).astype(CD_NP)

    def w3(W, sl):
        # w3[p, o, c] = W[o*128+p, c]
        return np.ascontiguousarray(
            W[:, sl].reshape(D // P, P, CPC).transpose(1, 0, 2)
        ).astype(CD_NP)

    in_maps = []
    for c in range(NCORES):
        sl = slice(c * CPC, (c + 1) * CPC)
        in_maps.append(
            {
                "x3": x3,
                "wq": w3(Wq, sl),
                "wk": w3(Wk, sl),
                "wv": w3(Wv, sl),
                "bq": np.ascontiguousarray(bq[sl].reshape(CPC, 1)),
                "bk": np.ascontiguousarray(bk[sl].reshape(CPC, 1)),
                "bv": np.ascontiguousarray(bv[sl].reshape(CPC, 1)),
            }
        )
    return in_maps


def host_epilogue(results, Wo, bo):
    """Normalize each core's unnormalized attention output and apply the
    output projection on host: y = sum_c (O_c / d_c).T @ Wo[c-slice] + bo."""
    on_full = np.empty((BS, D), np.float32)
    for c, r in enumerate(results):
        o = np.asarray(r["o_out"], np.float32)  # [130, BS]
        for h in range(2):
            num = o[h * (HD + 1) : h * (HD + 1) + HD, :]      # [64, BS]
            den = o[h * (HD + 1) + HD : h * (HD + 1) + HD + 1, :]  # [1, BS]
            on_full[:, (2 * c + h) * HD : (2 * c + h + 1) * HD] = (num / den).T
    y = on_full @ Wo.astype(np.float32) + bo.astype(np.float32)
    return y


def kernel(**inputs):
    global LAST_RESULTS
    Wo = np.asarray(inputs["Wo"], np.float32)
    bo = np.asarray(inputs["bo"], np.float32)
    nc = build_nc()
    in_maps = make_in_maps(inputs)
    res = bass_utils.run_bass_kernel_spmd(nc, in_maps, core_ids=list(range(NCORES)))
    LAST_RESULTS = res
    y = host_epilogue(res.results, Wo, bo)
    return y.reshape(B, S, D)
